# revision 2
# baseline (speedup 1.0000x reference)
"""Trainium2 Bass kernel for RecursiveMamba130M — fused-A formulation.

Math: with u = h @ WinT and y_i = sum_m G_m o u_{i-m}, the Hadamard with
G_m commutes into the output projection:
  z_i = y_i @ WoutT = sum_m h_{i-m} @ A_m,   A_m = WinT diag(G_m) WoutT.
A_m in R^{768x768} is a pure parameter transform (host, fp64). The device
does only 10 GEMMs of [128,768]x[768,768] (vs 8 of [128,768]x[768,1536]
in the two-GEMM form), and the whole u/y/acc vector pipeline vanishes.
Step embeddings enter the linear path via zb_j = sum_m s_{j-m} @ A_m
(host), seeded into PSUM with a K=1 matmul.

RMSNorm tail per loop: with q_z = sqrt(mean z^2 + eps), x'' = z + h*q_z
satisfies w = x''*rs_z and x_next = x''*c, c = rs_z*rs_w. rs_w is known
before w exists via sum w^2 = rs_z^2 sum z^2 + 2 rs_z sum zh + sum h^2
(sum zh from DVE mul+reduce overlapped with the ACT squares; sum h^2
precomputed off the critical path). The per-token scale c is applied
inside the PE "transpose" by a regular matmul against diag(c).

Scheduling notes (TimelineSim-derived):
- Readers of one PSUM tile serialize cross-engine in emission order, so
  z is SPLIT into zA [T,512] + zB [T,256] tiles; the side whose readers
  must run early is emitted to close first.
- The kernel is emitted as a FLAT schedule: each loop's norm-tail window
  is filled with hoisted GEMM terms of later loops whose operands are
  already resident (z rings are triple-buffered to license this).
- A 1-column matmul at ~1us starts the PE p-state ramp clock.
- qz/sw sqrt chains stay on ACT via the bias trick
  (sqrt(a/D + (b/D + eps)) with the parenthesis precomputed on ACT).

Sharding: data-parallel over the 1024 tokens (128/core, no collectives);
A_m replicated bf16.
"""

import numpy as np
import ml_dtypes

import concourse.bass as bass
import concourse.tile as tile
from concourse.bacc import Bacc
from concourse import masks, mybir
from concourse.bass_utils import run_bass_kernel_spmd

T = 128          # tokens per core
D = 768          # d_model
NL = 4           # reasoning loops
NCORES = 8
KCH = 6          # 128-row reduction chunks per GEMM
AW = KCH * D     # 4608 sbuf columns per A matrix
EPS = 1e-6

f32 = mybir.dt.float32
f32r = mybir.dt.float32r
bf16 = mybir.dt.bfloat16
AL = mybir.AluOpType
AF = mybir.ActivationFunctionType

_CACHE = {}


def build_nc():
    nc = Bacc()
    x0T_d = nc.dram_tensor("x0T", [128, KCH * T], f32, kind="ExternalInput")
    a_d = nc.dram_tensor("a_all", [128, NL * AW], bf16, kind="ExternalInput")
    zb_d = nc.dram_tensor("zb4", [1, NL * D], bf16, kind="ExternalInput")
    s4_d = nc.dram_tensor("s4", [NL, D], f32, kind="ExternalInput")
    s4T_d = nc.dram_tensor("s4T", [128, NL * KCH], bf16,
                           kind="ExternalInput")
    kc_d = nc.dram_tensor("kc4", [1, NL], f32, kind="ExternalInput")
    out_d = nc.dram_tensor("x_out", [T, D], f32, kind="ExternalOutput")

    with tile.TileContext(nc) as tc:
        with (
            tc.tile_pool(name="wpool", bufs=1) as wpool,
            tc.tile_pool(name="work", bufs=2) as work,
            tc.tile_pool(name="scal", bufs=1) as scal,
            tc.tile_pool(name="ps_a", bufs=3, space="PSUM") as ps_a,
            tc.tile_pool(name="ps_b", bufs=3, space="PSUM") as ps_b,
            tc.tile_pool(name="ps_t", bufs=2, space="PSUM") as ps_t,
        ):
            # ---------- constants ----------
            eps_t = scal.tile([T, 1], f32, tag="eps_t")
            nc.vector.memset(eps_t[:], EPS)
            act_pin = scal.tile([T, 1], f32, tag="act_pin")
            nc.scalar.activation(act_pin[:], eps_t[:], AF.Sqrt)

            # PE clock-ramp starter: a 1-col matmul as early as possible
            ramp_ps = ps_t.tile([T, 256], f32, tag="tp", name="ramp")
            nc.tensor.matmul(ramp_ps[0:1, 0:1], eps_t[:, 0:1], eps_t[:, 0:1],
                             start=True, stop=True)

            # Pool queue: identity first (gates the x0 transposes), then
            # the small [1,N] DMAs.
            ident_f = wpool.tile([128, 128], f32, tag="ident_f")
            masks.make_identity(nc, ident_f[:])
            ident_bf = wpool.tile([128, 128], bf16, tag="ident_bf")
            masks.make_identity(nc, ident_bf[:])
            s4_sb = wpool.tile([1, NL * D], f32r, tag="s4_sb")
            nc.gpsimd.dma_start(
                s4_sb[:].rearrange("o (m d) -> o m d", m=NL, d=D),
                s4_d[:, :].bitcast(f32r).rearrange("(o m) d -> o m d",
                                                   o=1, m=NL))
            zb_sb = wpool.tile([1, NL * D], bf16, tag="zb_sb")
            nc.gpsimd.dma_start(zb_sb[:], zb_d[0:1, :])
            s4T_sb = wpool.tile([128, NL * KCH], bf16, tag="s4T_sb")
            nc.gpsimd.dma_start(s4T_sb[:], s4T_d[:, :])
            kc_sb = wpool.tile([1, NL], f32r, tag="kc_sb")
            nc.gpsimd.dma_start(kc_sb[:], kc_d[0:1, :].bitcast(f32r))
            ones_bf = wpool.tile([1, 128], bf16, tag="ones_bf")
            nc.vector.memset(ones_bf[:], 1.0)
            ones_r = wpool.tile([1, 128], f32r, tag="ones_r")
            nc.vector.memset(ones_r[:].bitcast(mybir.dt.uint32), 0x3F800000)

            # ---------- big DMAs, ordered by first use ----------
            x0T_sb = wpool.tile([128, KCH * T], f32, tag="x0T_sb")
            nc.sync.dma_start(x0T_sb[:], x0T_d[:, :])

            a_sb = wpool.tile([128, NL * AW], bf16, tag="a_sb")

            def a_piece(m, k0, k1):
                lo, hi = m * AW + k0 * D, m * AW + k1 * D
                nc.sync.dma_start(a_sb[:, lo:hi], a_d[:, lo:hi])

            a_piece(0, 0, 2)
            a_piece(0, 2, 4)
            a_piece(0, 4, 6)
            a_piece(1, 0, 2)
            a_piece(1, 2, 4)
            a_piece(1, 4, 6)
            a_piece(2, 0, 3)
            a_piece(2, 3, 6)
            a_piece(3, 0, 3)
            a_piece(3, 3, 6)

            # ---------- Sb broadcasts (three [T,256] via ps_t ring) ------
            sb_all = wpool.tile([T, NL * D], f32, tag="sb_all")

            def emit_sb(jj):
                for c in range(3):
                    sbp = ps_t.tile([T, 256], f32, tag="tp",
                                    name=f"sb{jj}_{c}")
                    lo = D * jj + 256 * c
                    nc.tensor.matmul(sbp[:, :], ones_r[:, :],
                                     s4_sb[0:1, lo:lo + 256],
                                     start=True, stop=True)
                    if (jj + c) % 2 == 0:
                        nc.vector.tensor_copy(sb_all[:, lo:lo + 256], sbp[:])
                    else:
                        nc.scalar.copy(sb_all[:, lo:lo + 256], sbp[:])

            # ---------- x0 paths ----------
            x0Tb = wpool.tile([128, KCH * T], bf16, tag="x0Tb")
            nc.scalar.copy(x0Tb[:, 0:512], x0T_sb[:, 0:512])
            nc.scalar.copy(x0Tb[:, 512:768], x0T_sb[:, 512:768])
            # transpose x0T back to [t, d] and park in SBUF
            x0_sb = wpool.tile([T, D], f32, tag="x0_sb")
            for c in range(3):
                x0ps = ps_t.tile([T, 256], f32, tag="tp", name=f"x0r{c}")
                for cc in (0, 1):
                    k = 2 * c + cc
                    nc.tensor.transpose(
                        x0ps[:, 128 * cc:128 * (cc + 1)],
                        x0T_sb[:, 128 * k:128 * (k + 1)], ident_f[:])
                sl = slice(256 * c, 256 * (c + 1))
                if c == 1:
                    nc.vector.tensor_copy(x0_sb[:, sl], x0ps[:, :])
                else:
                    nc.scalar.copy(x0_sb[:, sl], x0ps[:, :])

            # ---------- schedule state ----------
            Z = {}                # j -> (zA, zB)
            xTb = [x0Tb]          # bf16 stationary operands per loop
            state = {"h": None, "xdd": None, "diag": None, "rsw": None}
            biasW = {}

            def alloc_z(j):
                Z[j] = (ps_a.tile([T, 512], f32, tag="zA", name=f"zA{j}"),
                        ps_b.tile([T, 256], f32, tag="zB", name=f"zB{j}"))

            def seed(j):
                zA, zB = Z[j]
                nc.tensor.matmul(zB[:, :], ones_bf[:, :],
                                 zb_sb[0:1, D * j + 512:D * j + 768],
                                 start=True, stop=False)
                nc.tensor.matmul(zA[:, :], ones_bf[:, :],
                                 zb_sb[0:1, D * j:D * j + 512],
                                 start=True, stop=False)

            def term(j, m, xi, last=False, a_first=False, ks=None):
                zA, zB = Z[j]
                base = m * AW
                kr = list(range(KCH)) if ks is None else list(ks)

                def side_b():
                    for k in kr:
                        nc.tensor.matmul(
                            zB[:, :],
                            xTb[xi][:, 128 * k:128 * (k + 1)],
                            a_sb[:, base + D * k + 512:base + D * k + 768],
                            start=False, stop=(last and k == KCH - 1))

                def side_a():
                    for k in kr:
                        nc.tensor.matmul(
                            zA[:, :],
                            xTb[xi][:, 128 * k:128 * (k + 1)],
                            a_sb[:, base + D * k:base + D * k + 512],
                            start=False, stop=(last and k == KCH - 1))

                if a_first:
                    side_a()
                    side_b()
                else:
                    side_b()
                    side_a()

            def tr_block(jj):
                """Transpose x''_{jj-1} with diag(c) into xT_jj; also emits
                ssh/biasW for loop jj (h_jj must be built already)."""
                xT_next = wpool.tile([128, KCH * T], bf16, tag=f"xT{jj}")
                for c in range(3):
                    trp = ps_t.tile([T, 256], f32, tag="tp",
                                    name=f"tr{jj}_{c}")
                    for cc in (0, 1):
                        k = 2 * c + cc
                        # regular matmul, rhs=diag(c): out[d, t] =
                        # xdd[t, d]*c[t] — a c-scaled transpose.
                        nc.tensor.matmul(
                            trp[:, 128 * cc:128 * (cc + 1)],
                            state["xdd"][:, 128 * k:128 * (k + 1)],
                            state["diag"][:], start=True, stop=True)
                    sl = slice(256 * c, 256 * (c + 1))
                    if c == 1:
                        nc.vector.tensor_copy(xT_next[:, sl], trp[:, :])
                    else:
                        nc.scalar.copy(xT_next[:, sl], trp[:, :])
                xTb.append(xT_next)
                # ssh path for loop jj without touching h:
                # sum h^2 = D(1 - eps*rsw^2) + 2*(x.s) + sum s^2, with the
                # x.s dot via 6 one-column matmuls against fresh xT.
                sx_ps = ps_t.tile([T, 256], f32, tag="tp", name=f"sx{jj}")
                for k in range(KCH):
                    nc.tensor.matmul(
                        sx_ps[:, 0:1],
                        xT_next[:, 128 * k:128 * (k + 1)],
                        s4T_sb[:, KCH * jj + k:KCH * jj + k + 1],
                        start=(k == 0), stop=(k == KCH - 1))
                e1 = scal.tile([T, 1], f32, tag=f"e1_{jj}")
                nc.vector.tensor_mul(e1[:], state["rsw"][:], state["rsw"][:])
                p1 = scal.tile([T, 1], f32, tag=f"p1_{jj}")
                nc.vector.scalar_tensor_tensor(
                    out=p1[:], in0=e1[:], scalar=-EPS,
                    in1=kcb_sb[:, jj:jj + 1], op0=AL.mult, op1=AL.add)
                bwn = scal.tile([T, 1], f32, tag=f"biasW{jj}")
                nc.vector.scalar_tensor_tensor(
                    out=bwn[:], in0=sx_ps[:, 0:1], scalar=2.0 / D,
                    in1=p1[:], op0=AL.mult, op1=AL.add)
                biasW[jj] = bwn

            def tail(j):
                zA, zB = Z[j]
                h = state["h"]
                zh_scr = work.tile([T, D], bf16, tag="zh_scr", bufs=2)
                sq_scr = work.tile([T, D], f32, tag="sq_scr", bufs=2)
                sszA = scal.tile([T, 1], f32, tag=f"sszA{j}")
                sszB = scal.tile([T, 1], f32, tag=f"sszB{j}")
                biasT = scal.tile([T, 1], f32, tag=f"biasT{j}")
                qz = scal.tile([T, 1], f32, tag=f"qz{j}")
                rsz = scal.tile([T, 1], f32, tag=f"rsz{j}")
                zh_acc = scal.tile([T, 1], f32, tag=f"zh{j}")
                t1 = scal.tile([T, 1], f32, tag=f"t1_{j}")
                s0 = scal.tile([T, 1], f32, tag=f"s0_{j}")
                t2 = scal.tile([T, 1], f32, tag=f"t2_{j}")
                t3 = scal.tile([T, 1], f32, tag=f"t3_{j}")
                t4 = scal.tile([T, 1], f32, tag=f"t4_{j}")
                d1 = scal.tile([T, 1], f32, tag=f"d1_{j}")
                dd = scal.tile([T, 1], f32, tag=f"dd_{j}")
                sw = scal.tile([T, 1], f32, tag=f"sw{j}")
                rsw = scal.tile([T, 1], f32, tag=f"rsw{j}")
                bw = biasW[j]

                def poly_a():
                    # first half: t2 = rsz^2 * sum z^2 (no sszph needed)
                    nc.vector.tensor_mul(t1[:], rsz[:], rsz[:])
                    nc.vector.tensor_add(s0[:], sszA[:], sszB[:])
                    nc.vector.tensor_mul(t2[:], t1[:], s0[:])

                def poly_b():
                    # t4 = t2 + 2*rsz*sum(zh) - rsz*D*(biasW-EPS) + ssh:
                    # classic identity, sum zh via DVE reduce (exact-ish;
                    # rsz is large here so zh needs f32-accurate handling)
                    nc.vector.tensor_mul(t3[:], rsz[:], zh_acc[:])
                    nc.vector.scalar_tensor_tensor(
                        out=t4[:], in0=t3[:], scalar=2.0, in1=t2[:],
                        op0=AL.mult, op1=AL.add)

                if j < NL - 1:
                    # zA closes early (a_first): sqA + zphA run pre-t0;
                    # sqB/biasT at t0 gate qz; sum(z+h)^2 on ACT.
                    nc.scalar.activation(sq_scr[:, 0:512], zA[:],
                                         AF.Square, accum_out=sszA[:])
                    nc.vector.tensor_mul(zh_scr[:, 0:512], zA[:],
                                         h[:, 0:512])
                    nc.scalar.activation(sq_scr[:, 512:768], zB[:],
                                         AF.Square, accum_out=sszB[:])
                    nc.scalar.activation(biasT[:], sszB[:], AF.Copy,
                                         scale=1.0 / D, bias=EPS)
                    nc.scalar.activation(qz[:], sszA[:], AF.Sqrt,
                                         bias=biasT[:, :], scale=1.0 / D)
                    nc.vector.tensor_mul(zh_scr[:, 512:768], zB[:],
                                         h[:, 512:768])
                    nc.vector.tensor_reduce(zh_acc[:], zh_scr[:],
                                            mybir.AxisListType.X, AL.add)
                    nc.vector.reciprocal(rsz[:], qz[:])
                    poly_a()
                    # x'' = z + h*qz, in bf16 (feeds 1-cyc/row transposes)
                    xdd = work.tile([T, D], bf16, tag="xdd", bufs=2)
                    nc.vector.scalar_tensor_tensor(
                        out=xdd[:, 0:512], in0=h[:, 0:512], scalar=qz[:, :],
                        in1=zA[:, :], op0=AL.mult, op1=AL.add)
                    poly_b()
                    nc.scalar.activation(sw[:], t4[:], AF.Sqrt,
                                         bias=bw[:, :], scale=1.0 / D)
                    nc.vector.reciprocal(rsw[:], sw[:])
                    c_col = scal.tile([T, 1], f32, tag=f"c{j}")
                    nc.vector.tensor_mul(c_col[:], rsz[:], rsw[:])
                    diag_c = work.tile([128, 128], bf16, tag="diag", bufs=2)
                    nc.vector.tensor_scalar_mul(diag_c[:], ident_bf[:],
                                                c_col[:, :])
                    nc.vector.scalar_tensor_tensor(
                        out=xdd[:, 512:768], in0=h[:, 512:768],
                        scalar=qz[:, :],
                        in1=zB[:, :], op0=AL.mult, op1=AL.add)
                    # h_{j+1} = x''*c + s_{j+1} in one DVE stt/chunk

                    h_next = work.tile([T, D], f32, tag="h", bufs=2)
                    for c in range(3):
                        sl = slice(256 * c, 256 * (c + 1))
                        nc.vector.scalar_tensor_tensor(
                            out=h_next[:, sl], in0=xdd[:, sl],
                            scalar=c_col[:, :],
                            in1=sb_all[:, D * (j + 1) + 256 * c:
                                       D * (j + 1) + 256 * (c + 1)],
                            op0=AL.mult, op1=AL.add)
                    state["h"] = h_next
                    state["xdd"] = xdd
                    state["diag"] = diag_c
                    state["rsw"] = rsw
                else:
                    # endgame: zA closed early (a_first): sqA (qz gate)
                    # first, then zhA; short qz chain, w on DVE, scales
                    # split ACT/DVE, chunked output DMA.
                    nc.scalar.activation(sq_scr[:, 0:512], zA[:],
                                         AF.Square, accum_out=sszA[:])
                    nc.vector.tensor_mul(zh_scr[:, 0:512], zA[:],
                                         h[:, 0:512])
                    nc.scalar.activation(sq_scr[:, 512:768], zB[:],
                                         AF.Square, accum_out=sszB[:])
                    nc.scalar.activation(biasT[:], sszB[:], AF.Copy,
                                         scale=1.0 / D, bias=EPS)
                    nc.scalar.activation(qz[:], sszA[:], AF.Sqrt,
                                         bias=biasT[:, :], scale=1.0 / D)
                    nc.vector.tensor_mul(zh_scr[:, 512:768], zB[:],
                                         h[:, 512:768])
                    nc.vector.tensor_reduce(zh_acc[:], zh_scr[:],
                                            mybir.AxisListType.X, AL.add)
                    nc.vector.reciprocal(rsz[:], qz[:])
                    poly_a()
                    w_t = work.tile([T, D], f32, tag="w_t", bufs=1)
                    nc.vector.scalar_tensor_tensor(
                        out=w_t[:, 0:384], in0=zA[:, 0:384], scalar=rsz[:, :],
                        in1=h[:, 0:384], op0=AL.mult, op1=AL.add)
                    poly_b()
                    nc.scalar.activation(sw[:], t4[:], AF.Sqrt,
                                         bias=bw[:, :], scale=1.0 / D)
                    nc.vector.scalar_tensor_tensor(
                        out=w_t[:, 384:512], in0=zA[:, 384:512],
                        scalar=rsz[:, :],
                        in1=h[:, 384:512], op0=AL.mult, op1=AL.add)
                    nc.vector.reciprocal(rsw[:], sw[:])
                    nc.vector.scalar_tensor_tensor(
                        out=w_t[:, 512:768], in0=zB[:, :], scalar=rsz[:, :],
                        in1=h[:, 512:768], op0=AL.mult, op1=AL.add)
                    out_t = work.tile([T, D], f32, tag="out_t", bufs=1)
                    nc.scalar.activation(out_t[:, 0:384], w_t[:, 0:384],
                                         AF.Copy, scale=rsw[:, :])
                    nc.sync.dma_start(out_d[:, 0:384], out_t[:, 0:384])
                    nc.vector.tensor_scalar_mul(out_t[:, 384:768],
                                                w_t[:, 384:768], rsw[:, :])
                    nc.sync.dma_start(out_d[:, 384:768], out_t[:, 384:768])

            # ---------- flat schedule ----------
            # loop 0
            alloc_z(0)
            seed(0)
            term(0, 0, 0, last=True)
            emit_sb(0)
            h0 = work.tile([T, D], f32, tag="h", bufs=2)
            for c in range(3):
                sl = slice(256 * c, 256 * (c + 1))
                nc.gpsimd.tensor_add(h0[:, sl], x0_sb[:, sl], sb_all[:, sl])
            state["h"] = h0
            # kcb[t, j] = (D + sum s_j^2)/D + eps broadcast per token
            kcb_ps = ps_t.tile([T, 256], f32, tag="tp", name="kcb")
            nc.tensor.matmul(kcb_ps[:, 0:NL], ones_r[:, :], kc_sb[0:1, :],
                             start=True, stop=True)
            kcb_sb = wpool.tile([T, NL], f32, tag="kcb_sb")
            nc.vector.tensor_copy(kcb_sb[:], kcb_ps[:, 0:NL])
            ssh0 = scal.tile([T, 1], f32, tag="ssh0")
            hsq0 = work.tile([T, D], f32, tag="hsq", bufs=2)
            nc.scalar.activation(hsq0[:], h0[:], AF.Square, accum_out=ssh0[:])
            bw0 = scal.tile([T, 1], f32, tag="biasW0")
            nc.scalar.activation(bw0[:], ssh0[:], AF.Copy, scale=1.0 / D,
                                 bias=EPS)
            biasW[0] = bw0
            emit_sb(1)
            tail(0)

            # loop 1: x0@A_1 is DMA-paced and hides tail-0
            alloc_z(1)
            seed(1)
            term(1, 1, 0)                      # x0 @ A_1 (streaming)
            alloc_z(2)
            seed(2)
            emit_sb(2)
            tr_block(1)
            term(1, 0, 1, last=True, a_first=True)           # x1 @ A_0 -> t0_1
            tail(1)

            # loop 2: resident-A terms hide tail-1
            term(2, 2, 0)                      # x0 @ A_2
            term(2, 1, 1)                      # x1 @ A_1
            emit_sb(3)
            tr_block(2)
            term(2, 0, 2, last=True, a_first=True)           # x2 @ A_0 -> t0_2
            tail(2)

            # loop 3: resident terms hide tail-2
            alloc_z(3)
            seed(3)
            term(3, 2, 1)                      # x1 @ A_2
            term(3, 1, 2)                      # x2 @ A_1
            tr_block(3)
            term(3, 0, 3)                      # x3 @ A_0
            term(3, 3, 0, last=True, a_first=True)   # x0 @ A_3 (last DMA)
            tail(3)

    nc.compile()
    return nc


def _host_prep(in_proj_base, lora_A, lora_B, A_theta, B_real, B_imag,
               C_real, C_imag, out_proj_w, step_emb):
    W = in_proj_base.astype(np.float64) + 2.0 * (
        lora_B.astype(np.float64) @ lora_A.astype(np.float64))   # [2d, d]
    WinT = np.ascontiguousarray(W.T)                             # [768, 1536]
    WoutT = out_proj_w.astype(np.float64).T                      # [1536, 768]

    th = A_theta.astype(np.float64)
    P = (C_real.astype(np.float64) * B_real.astype(np.float64)
         - C_imag.astype(np.float64) * B_imag.astype(np.float64))
    Q = (C_real.astype(np.float64) * B_imag.astype(np.float64)
         + C_imag.astype(np.float64) * B_real.astype(np.float64))
    g4 = np.stack([
        (P * np.cos(m * th) - Q * np.sin(m * th)).sum(-1).reshape(-1)
        for m in range(NL)
    ])                                                           # [4, 1536]

    A = np.stack([WinT @ (g4[m][:, None] * WoutT) for m in range(NL)])
    s = step_emb.astype(np.float64)
    zb = np.stack([
        sum(s[jj - m] @ A[m] for m in range(jj + 1)) for jj in range(NL)
    ])                                                           # [4, 768]

    # device layout [128, NL, KCH, D]: elem [p][m,k,n] = A[m, 128k+p, n]
    a_dev = np.ascontiguousarray(
        A.reshape(NL, KCH, 128, D).transpose(2, 0, 1, 3)
        .reshape(128, NL * KCH * D)).astype(ml_dtypes.bfloat16)
    zb_dev = zb.reshape(1, NL * D).astype(ml_dtypes.bfloat16)
    s_dev = np.ascontiguousarray(step_emb).astype(np.float32)
    s4T_dev = np.ascontiguousarray(
        s_dev.reshape(NL, KCH, 128).transpose(2, 0, 1)
        .reshape(128, NL * KCH)).astype(ml_dtypes.bfloat16)
    kc_dev = np.array([[
        (D + float((s_dev[jj].astype(np.float64) ** 2).sum())) / D + EPS
        for jj in range(NL)
    ]], dtype=np.float32)
    return a_dev, zb_dev, s_dev, s4T_dev, kc_dev


def kernel(x, in_proj_base, lora_A, lora_B, A_theta, B_real, B_imag,
           C_real, C_imag, out_proj_w, mixer_norm_w, loop_norm_w, step_emb,
           _trace=False):
    x = np.asarray(x, dtype=np.float32)
    a_dev, zb_dev, s_dev, s4T_dev, kc_dev = _host_prep(
        np.asarray(in_proj_base), np.asarray(lora_A), np.asarray(lora_B),
        np.asarray(A_theta), np.asarray(B_real), np.asarray(B_imag),
        np.asarray(C_real), np.asarray(C_imag), np.asarray(out_proj_w),
        np.asarray(step_emb))
    # mixer_norm_w / loop_norm_w are ones per the problem spec; rmsnorm
    # weight multiplies are identity and omitted on device.

    if "nc" not in _CACHE:
        _CACHE["nc"] = build_nc()
    nc = _CACHE["nc"]

    shared = {"a_all": a_dev, "zb4": zb_dev, "s4": s_dev,
              "s4T": s4T_dev, "kc4": kc_dev}
    in_maps = []
    for c in range(NCORES):
        xs = np.ascontiguousarray(x[0, T * c:T * (c + 1), :])    # [128, 768]
        x0T = np.ascontiguousarray(
            xs.T.reshape(KCH, 128, T).transpose(1, 0, 2)
            .reshape(128, KCH * T)).astype(np.float32)
        in_maps.append({**shared, "x0T": x0T})
    res = run_bass_kernel_spmd(nc, in_maps, list(range(NCORES)), trace=_trace)
    out = np.concatenate(
        [np.asarray(res.results[c]["x_out"]) for c in range(NCORES)], axis=0)
    if _trace:
        _CACHE["last_result"] = res
    return out[None, :, :].astype(np.float32)


# revision 3
# speedup vs baseline: 1.0002x; 1.0002x over previous
"""Trainium2 Bass kernel for RecursiveMamba130M — fused-A formulation.

Math: with u = h @ WinT and y_i = sum_m G_m o u_{i-m}, the Hadamard with
G_m commutes into the output projection:
  z_i = y_i @ WoutT = sum_m h_{i-m} @ A_m,   A_m = WinT diag(G_m) WoutT.
A_m in R^{768x768} is a pure parameter transform (host, fp64). The device
does only 10 GEMMs of [128,768]x[768,768] (vs 8 of [128,768]x[768,1536]
in the two-GEMM form), and the whole u/y/acc vector pipeline vanishes.
Step embeddings enter the linear path via zb_j = sum_m s_{j-m} @ A_m
(host), seeded into PSUM with a K=1 matmul.

RMSNorm tail per loop: with q_z = sqrt(mean z^2 + eps), x'' = z + h*q_z
satisfies w = x''*rs_z and x_next = x''*c, c = rs_z*rs_w. rs_w is known
before w exists via sum w^2 = rs_z^2 sum z^2 + 2 rs_z sum zh + sum h^2
(sum zh from DVE mul+reduce overlapped with the ACT squares; sum h^2
precomputed off the critical path). The per-token scale c is applied
inside the PE "transpose" by a regular matmul against diag(c).

Scheduling notes (TimelineSim-derived):
- Readers of one PSUM tile serialize cross-engine in emission order, so
  z is SPLIT into zA [T,512] + zB [T,256] tiles; the side whose readers
  must run early is emitted to close first.
- The kernel is emitted as a FLAT schedule: each loop's norm-tail window
  is filled with hoisted GEMM terms of later loops whose operands are
  already resident (z rings are triple-buffered to license this).
- A 1-column matmul at ~1us starts the PE p-state ramp clock.
- qz/sw sqrt chains stay on ACT via the bias trick
  (sqrt(a/D + (b/D + eps)) with the parenthesis precomputed on ACT).

Sharding: data-parallel over the 1024 tokens (128/core, no collectives);
A_m replicated bf16.
"""

import numpy as np
import ml_dtypes

import concourse.bass as bass
import concourse.tile as tile
from concourse.bacc import Bacc
from concourse import masks, mybir
from concourse.bass_utils import run_bass_kernel_spmd

T = 128          # tokens per core
D = 768          # d_model
NL = 4           # reasoning loops
NCORES = 8
KCH = 6          # 128-row reduction chunks per GEMM
AW = KCH * D     # 4608 sbuf columns per A matrix
EPS = 1e-6

f32 = mybir.dt.float32
f32r = mybir.dt.float32r
bf16 = mybir.dt.bfloat16
AL = mybir.AluOpType
AF = mybir.ActivationFunctionType

_CACHE = {}


def build_nc():
    nc = Bacc()
    x0T_d = nc.dram_tensor("x0T", [128, KCH * T], f32, kind="ExternalInput")
    a_d = nc.dram_tensor("a_all", [128, NL * AW], bf16, kind="ExternalInput")
    zb_d = nc.dram_tensor("zb4", [1, NL * D], bf16, kind="ExternalInput")
    s4_d = nc.dram_tensor("s4", [NL, D], f32, kind="ExternalInput")
    s4T_d = nc.dram_tensor("s4T", [128, NL * KCH], bf16,
                           kind="ExternalInput")
    kc_d = nc.dram_tensor("kc4", [1, NL], f32, kind="ExternalInput")
    out_d = nc.dram_tensor("x_out", [T, D], f32, kind="ExternalOutput")

    with tile.TileContext(nc) as tc:
        with (
            tc.tile_pool(name="wpool", bufs=1) as wpool,
            tc.tile_pool(name="work", bufs=2) as work,
            tc.tile_pool(name="scal", bufs=1) as scal,
            tc.tile_pool(name="ps_a", bufs=3, space="PSUM") as ps_a,
            tc.tile_pool(name="ps_b", bufs=3, space="PSUM") as ps_b,
            tc.tile_pool(name="ps_t", bufs=2, space="PSUM") as ps_t,
        ):
            # ---------- constants ----------
            eps_t = scal.tile([T, 1], f32, tag="eps_t")
            nc.vector.memset(eps_t[:], EPS)
            act_pin = scal.tile([T, 1], f32, tag="act_pin")
            nc.scalar.activation(act_pin[:], eps_t[:], AF.Sqrt)

            # PE clock-ramp starter: a 1-col matmul as early as possible
            ramp_ps = ps_t.tile([T, 256], f32, tag="tp", name="ramp")
            nc.tensor.matmul(ramp_ps[0:1, 0:1], eps_t[:, 0:1], eps_t[:, 0:1],
                             start=True, stop=True)

            # Pool queue: identity first (gates the x0 transposes), then
            # the small [1,N] DMAs.
            ident_f = wpool.tile([128, 128], f32, tag="ident_f")
            masks.make_identity(nc, ident_f[:])
            ident_bf = wpool.tile([128, 128], bf16, tag="ident_bf")
            masks.make_identity(nc, ident_bf[:])
            s4_sb = wpool.tile([1, NL * D], f32r, tag="s4_sb")
            nc.gpsimd.dma_start(
                s4_sb[:].rearrange("o (m d) -> o m d", m=NL, d=D),
                s4_d[:, :].bitcast(f32r).rearrange("(o m) d -> o m d",
                                                   o=1, m=NL))
            zb_sb = wpool.tile([1, NL * D], bf16, tag="zb_sb")
            nc.gpsimd.dma_start(zb_sb[:], zb_d[0:1, :])
            s4T_sb = wpool.tile([128, NL * KCH], bf16, tag="s4T_sb")
            nc.gpsimd.dma_start(s4T_sb[:], s4T_d[:, :])
            kc_sb = wpool.tile([1, NL], f32r, tag="kc_sb")
            nc.gpsimd.dma_start(kc_sb[:], kc_d[0:1, :].bitcast(f32r))
            ones_bf = wpool.tile([1, 128], bf16, tag="ones_bf")
            nc.vector.memset(ones_bf[:], 1.0)
            ones_r = wpool.tile([1, 128], f32r, tag="ones_r")
            nc.vector.memset(ones_r[:].bitcast(mybir.dt.uint32), 0x3F800000)

            # ---------- big DMAs, ordered by first use ----------
            x0T_sb = wpool.tile([128, KCH * T], f32, tag="x0T_sb")
            nc.sync.dma_start(x0T_sb[:], x0T_d[:, :])

            a_sb = wpool.tile([128, NL * AW], bf16, tag="a_sb")

            def a_piece(m, k0, k1):
                lo, hi = m * AW + k0 * D, m * AW + k1 * D
                nc.sync.dma_start(a_sb[:, lo:hi], a_d[:, lo:hi])

            a_piece(0, 0, 2)
            a_piece(0, 2, 4)
            a_piece(0, 4, 6)
            a_piece(1, 0, 2)
            a_piece(1, 2, 4)
            a_piece(1, 4, 6)
            a_piece(2, 0, 3)
            a_piece(2, 3, 6)
            a_piece(3, 0, 3)
            a_piece(3, 3, 6)

            # ---------- Sb broadcasts (three [T,256] via ps_t ring) ------
            sb_all = wpool.tile([T, NL * D], f32, tag="sb_all")

            def emit_sb(jj):
                for c in range(3):
                    sbp = ps_t.tile([T, 256], f32, tag="tp",
                                    name=f"sb{jj}_{c}")
                    lo = D * jj + 256 * c
                    nc.tensor.matmul(sbp[:, :], ones_r[:, :],
                                     s4_sb[0:1, lo:lo + 256],
                                     start=True, stop=True)
                    if (jj + c) % 2 == 0:
                        nc.vector.tensor_copy(sb_all[:, lo:lo + 256], sbp[:])
                    else:
                        nc.scalar.copy(sb_all[:, lo:lo + 256], sbp[:])

            # ---------- x0 paths ----------
            x0Tb = wpool.tile([128, KCH * T], bf16, tag="x0Tb")
            nc.scalar.copy(x0Tb[:, 0:512], x0T_sb[:, 0:512])
            nc.scalar.copy(x0Tb[:, 512:768], x0T_sb[:, 512:768])
            # transpose x0T back to [t, d] and park in SBUF
            x0_sb = wpool.tile([T, D], f32, tag="x0_sb")
            for c in range(3):
                x0ps = ps_t.tile([T, 256], f32, tag="tp", name=f"x0r{c}")
                for cc in (0, 1):
                    k = 2 * c + cc
                    nc.tensor.transpose(
                        x0ps[:, 128 * cc:128 * (cc + 1)],
                        x0T_sb[:, 128 * k:128 * (k + 1)], ident_f[:])
                sl = slice(256 * c, 256 * (c + 1))
                if c == 1:
                    nc.vector.tensor_copy(x0_sb[:, sl], x0ps[:, :])
                else:
                    nc.scalar.copy(x0_sb[:, sl], x0ps[:, :])

            # ---------- schedule state ----------
            Z = {}                # j -> (zA, zB)
            xTb = [x0Tb]          # bf16 stationary operands per loop
            state = {"h": None, "xdd": None, "diag": None, "rsw": None}
            biasW = {}

            def alloc_z(j):
                Z[j] = (ps_a.tile([T, 512], f32, tag="zA", name=f"zA{j}"),
                        ps_b.tile([T, 256], f32, tag="zB", name=f"zB{j}"))

            def seed(j):
                zA, zB = Z[j]
                nc.tensor.matmul(zB[:, :], ones_bf[:, :],
                                 zb_sb[0:1, D * j + 512:D * j + 768],
                                 start=True, stop=False)
                nc.tensor.matmul(zA[:, :], ones_bf[:, :],
                                 zb_sb[0:1, D * j:D * j + 512],
                                 start=True, stop=False)

            def term(j, m, xi, last=False, a_first=False, ks=None):
                zA, zB = Z[j]
                base = m * AW
                kr = list(range(KCH)) if ks is None else list(ks)

                def side_b():
                    for k in kr:
                        nc.tensor.matmul(
                            zB[:, :],
                            xTb[xi][:, 128 * k:128 * (k + 1)],
                            a_sb[:, base + D * k + 512:base + D * k + 768],
                            start=False, stop=(last and k == KCH - 1))

                def side_a():
                    for k in kr:
                        nc.tensor.matmul(
                            zA[:, :],
                            xTb[xi][:, 128 * k:128 * (k + 1)],
                            a_sb[:, base + D * k:base + D * k + 512],
                            start=False, stop=(last and k == KCH - 1))

                if a_first:
                    side_a()
                    side_b()
                else:
                    side_b()
                    side_a()

            def tr_block(jj):
                """Transpose x''_{jj-1} with diag(c) into xT_jj; also emits
                ssh/biasW for loop jj (h_jj must be built already)."""
                xT_next = wpool.tile([128, KCH * T], bf16, tag=f"xT{jj}")
                for c in range(3):
                    trp = ps_t.tile([T, 256], f32, tag="tp",
                                    name=f"tr{jj}_{c}")
                    for cc in (0, 1):
                        k = 2 * c + cc
                        # regular matmul, rhs=diag(c): out[d, t] =
                        # xdd[t, d]*c[t] — a c-scaled transpose.
                        nc.tensor.matmul(
                            trp[:, 128 * cc:128 * (cc + 1)],
                            state["xdd"][:, 128 * k:128 * (k + 1)],
                            state["diag"][:], start=True, stop=True)
                    sl = slice(256 * c, 256 * (c + 1))
                    if c == 1:
                        nc.vector.tensor_copy(xT_next[:, sl], trp[:, :])
                    else:
                        nc.scalar.copy(xT_next[:, sl], trp[:, :])
                xTb.append(xT_next)
                # ssh path for loop jj without touching h:
                # sum h^2 = D(1 - eps*rsw^2) + 2*(x.s) + sum s^2, with the
                # x.s dot via 6 one-column matmuls against fresh xT.
                sx_ps = ps_t.tile([T, 256], f32, tag="tp", name=f"sx{jj}")
                for k in range(KCH):
                    nc.tensor.matmul(
                        sx_ps[:, 0:1],
                        xT_next[:, 128 * k:128 * (k + 1)],
                        s4T_sb[:, KCH * jj + k:KCH * jj + k + 1],
                        start=(k == 0), stop=(k == KCH - 1))
                e1 = scal.tile([T, 1], f32, tag=f"e1_{jj}")
                nc.vector.tensor_mul(e1[:], state["rsw"][:], state["rsw"][:])
                p1 = scal.tile([T, 1], f32, tag=f"p1_{jj}")
                nc.vector.scalar_tensor_tensor(
                    out=p1[:], in0=e1[:], scalar=-EPS,
                    in1=kcb_sb[:, jj:jj + 1], op0=AL.mult, op1=AL.add)
                bwn = scal.tile([T, 1], f32, tag=f"biasW{jj}")
                nc.vector.scalar_tensor_tensor(
                    out=bwn[:], in0=sx_ps[:, 0:1], scalar=2.0 / D,
                    in1=p1[:], op0=AL.mult, op1=AL.add)
                biasW[jj] = bwn

            def tail(j):
                zA, zB = Z[j]
                h = state["h"]
                zh_scr = work.tile([T, D], bf16, tag="zh_scr", bufs=2)
                sq_scr = work.tile([T, D], f32, tag="sq_scr", bufs=2)
                sszA = scal.tile([T, 1], f32, tag=f"sszA{j}")
                sszB = scal.tile([T, 1], f32, tag=f"sszB{j}")
                biasT = scal.tile([T, 1], f32, tag=f"biasT{j}")
                qz = scal.tile([T, 1], f32, tag=f"qz{j}")
                rsz = scal.tile([T, 1], f32, tag=f"rsz{j}")
                zh_acc = scal.tile([T, 1], f32, tag=f"zh{j}")
                t1 = scal.tile([T, 1], f32, tag=f"t1_{j}")
                s0 = scal.tile([T, 1], f32, tag=f"s0_{j}")
                t2 = scal.tile([T, 1], f32, tag=f"t2_{j}")
                t3 = scal.tile([T, 1], f32, tag=f"t3_{j}")
                t4 = scal.tile([T, 1], f32, tag=f"t4_{j}")
                d1 = scal.tile([T, 1], f32, tag=f"d1_{j}")
                dd = scal.tile([T, 1], f32, tag=f"dd_{j}")
                sw = scal.tile([T, 1], f32, tag=f"sw{j}")
                rsw = scal.tile([T, 1], f32, tag=f"rsw{j}")
                bw = biasW[j]

                def poly_a():
                    # first half: t2 = rsz^2 * sum z^2 (no sszph needed)
                    nc.vector.tensor_mul(t1[:], rsz[:], rsz[:])
                    nc.vector.tensor_add(s0[:], sszA[:], sszB[:])
                    nc.vector.tensor_mul(t2[:], t1[:], s0[:])

                def poly_b():
                    # t4 = t2 + 2*rsz*sum(zh) - rsz*D*(biasW-EPS) + ssh:
                    # classic identity, sum zh via DVE reduce (exact-ish;
                    # rsz is large here so zh needs f32-accurate handling)
                    nc.vector.tensor_mul(t3[:], rsz[:], zh_acc[:])
                    nc.vector.scalar_tensor_tensor(
                        out=t4[:], in0=t3[:], scalar=2.0, in1=t2[:],
                        op0=AL.mult, op1=AL.add)

                if j < NL - 1:
                    # zA closes early (a_first): sqA + zphA run pre-t0;
                    # sqB/biasT at t0 gate qz; sum(z+h)^2 on ACT.
                    nc.scalar.activation(sq_scr[:, 0:512], zA[:],
                                         AF.Square, accum_out=sszA[:])
                    nc.vector.tensor_mul(zh_scr[:, 0:512], zA[:],
                                         h[:, 0:512])
                    nc.scalar.activation(sq_scr[:, 512:768], zB[:],
                                         AF.Square, accum_out=sszB[:])
                    nc.scalar.activation(biasT[:], sszB[:], AF.Copy,
                                         scale=1.0 / D, bias=EPS)
                    nc.scalar.activation(qz[:], sszA[:], AF.Sqrt,
                                         bias=biasT[:, :], scale=1.0 / D)
                    nc.vector.tensor_mul(zh_scr[:, 512:768], zB[:],
                                         h[:, 512:768])
                    nc.vector.tensor_reduce(zh_acc[:], zh_scr[:],
                                            mybir.AxisListType.X, AL.add)
                    nc.vector.reciprocal(rsz[:], qz[:])
                    poly_a()
                    # x'' = z + h*qz, in bf16 (feeds 1-cyc/row transposes)
                    xdd = work.tile([T, D], bf16, tag="xdd", bufs=2)
                    nc.vector.scalar_tensor_tensor(
                        out=xdd[:, 0:512], in0=h[:, 0:512], scalar=qz[:, :],
                        in1=zA[:, :], op0=AL.mult, op1=AL.add)
                    poly_b()
                    nc.scalar.activation(sw[:], t4[:], AF.Sqrt,
                                         bias=bw[:, :], scale=1.0 / D)
                    nc.vector.reciprocal(rsw[:], sw[:])
                    c_col = scal.tile([T, 1], f32, tag=f"c{j}")
                    nc.vector.tensor_mul(c_col[:], rsz[:], rsw[:])
                    diag_c = work.tile([128, 128], bf16, tag="diag", bufs=2)
                    nc.vector.tensor_scalar_mul(diag_c[:], ident_bf[:],
                                                c_col[:, :])
                    nc.vector.scalar_tensor_tensor(
                        out=xdd[:, 512:768], in0=h[:, 512:768],
                        scalar=qz[:, :],
                        in1=zB[:, :], op0=AL.mult, op1=AL.add)
                    # h_{j+1} = x''*c + s_{j+1} in one DVE stt/chunk

                    h_next = work.tile([T, D], f32, tag="h", bufs=2)
                    for c in range(3):
                        sl = slice(256 * c, 256 * (c + 1))
                        nc.vector.scalar_tensor_tensor(
                            out=h_next[:, sl], in0=xdd[:, sl],
                            scalar=c_col[:, :],
                            in1=sb_all[:, D * (j + 1) + 256 * c:
                                       D * (j + 1) + 256 * (c + 1)],
                            op0=AL.mult, op1=AL.add)
                    state["h"] = h_next
                    state["xdd"] = xdd
                    state["diag"] = diag_c
                    state["rsw"] = rsw
                else:
                    # endgame: zA closed early (a_first): sqA (qz gate)
                    # first, then zhA; short qz chain, w on DVE, scales
                    # split ACT/DVE, chunked output DMA.
                    nc.scalar.activation(sq_scr[:, 0:512], zA[:],
                                         AF.Square, accum_out=sszA[:])
                    nc.vector.tensor_mul(zh_scr[:, 0:512], zA[:],
                                         h[:, 0:512])
                    nc.scalar.activation(sq_scr[:, 512:768], zB[:],
                                         AF.Square, accum_out=sszB[:])
                    nc.scalar.activation(biasT[:], sszB[:], AF.Copy,
                                         scale=1.0 / D, bias=EPS)
                    nc.scalar.activation(qz[:], sszA[:], AF.Sqrt,
                                         bias=biasT[:, :], scale=1.0 / D)
                    nc.vector.tensor_mul(zh_scr[:, 512:768], zB[:],
                                         h[:, 512:768])
                    nc.vector.tensor_reduce(zh_acc[:], zh_scr[:],
                                            mybir.AxisListType.X, AL.add)
                    nc.vector.reciprocal(rsz[:], qz[:])
                    poly_a()
                    w_t = work.tile([T, D], f32, tag="w_t", bufs=1)
                    nc.vector.scalar_tensor_tensor(
                        out=w_t[:, 0:384], in0=zA[:, 0:384], scalar=rsz[:, :],
                        in1=h[:, 0:384], op0=AL.mult, op1=AL.add)
                    poly_b()
                    nc.scalar.activation(sw[:], t4[:], AF.Sqrt,
                                         bias=bw[:, :], scale=1.0 / D)
                    nc.vector.scalar_tensor_tensor(
                        out=w_t[:, 384:512], in0=zA[:, 384:512],
                        scalar=rsz[:, :],
                        in1=h[:, 384:512], op0=AL.mult, op1=AL.add)
                    nc.vector.reciprocal(rsw[:], sw[:])
                    nc.vector.scalar_tensor_tensor(
                        out=w_t[:, 512:768], in0=zB[:, :], scalar=rsz[:, :],
                        in1=h[:, 512:768], op0=AL.mult, op1=AL.add)
                    out_t = work.tile([T, D], f32, tag="out_t", bufs=1)
                    nc.scalar.activation(out_t[:, 0:384], w_t[:, 0:384],
                                         AF.Copy, scale=rsw[:, :])
                    nc.sync.dma_start(out_d[:, 0:384], out_t[:, 0:384])
                    nc.vector.tensor_scalar_mul(out_t[:, 384:768],
                                                w_t[:, 384:768], rsw[:, :])
                    nc.scalar.dma_start(out_d[:, 384:768],
                                        out_t[:, 384:768])

            # ---------- flat schedule ----------
            # loop 0
            alloc_z(0)
            seed(0)
            term(0, 0, 0, last=True)
            emit_sb(0)
            h0 = work.tile([T, D], f32, tag="h", bufs=2)
            for c in range(3):
                sl = slice(256 * c, 256 * (c + 1))
                nc.gpsimd.tensor_add(h0[:, sl], x0_sb[:, sl], sb_all[:, sl])
            state["h"] = h0
            # kcb[t, j] = (D + sum s_j^2)/D + eps broadcast per token
            kcb_ps = ps_t.tile([T, 256], f32, tag="tp", name="kcb")
            nc.tensor.matmul(kcb_ps[:, 0:NL], ones_r[:, :], kc_sb[0:1, :],
                             start=True, stop=True)
            kcb_sb = wpool.tile([T, NL], f32, tag="kcb_sb")
            nc.vector.tensor_copy(kcb_sb[:], kcb_ps[:, 0:NL])
            ssh0 = scal.tile([T, 1], f32, tag="ssh0")
            hsq0 = work.tile([T, D], f32, tag="hsq", bufs=2)
            nc.scalar.activation(hsq0[:], h0[:], AF.Square, accum_out=ssh0[:])
            bw0 = scal.tile([T, 1], f32, tag="biasW0")
            nc.scalar.activation(bw0[:], ssh0[:], AF.Copy, scale=1.0 / D,
                                 bias=EPS)
            biasW[0] = bw0
            emit_sb(1)
            tail(0)

            # loop 1: x0@A_1 is DMA-paced and hides tail-0
            alloc_z(1)
            seed(1)
            term(1, 1, 0)                      # x0 @ A_1 (streaming)
            alloc_z(2)
            seed(2)
            emit_sb(2)
            tr_block(1)
            term(1, 0, 1, last=True, a_first=True)           # x1 @ A_0 -> t0_1
            tail(1)

            # loop 2: resident-A terms hide tail-1
            term(2, 2, 0)                      # x0 @ A_2
            term(2, 1, 1)                      # x1 @ A_1
            emit_sb(3)
            tr_block(2)
            term(2, 0, 2, last=True, a_first=True)           # x2 @ A_0 -> t0_2
            tail(2)

            # loop 3: resident terms hide tail-2
            alloc_z(3)
            seed(3)
            term(3, 2, 1)                      # x1 @ A_2
            term(3, 1, 2)                      # x2 @ A_1
            tr_block(3)
            term(3, 0, 3)                      # x3 @ A_0
            term(3, 3, 0, last=True, a_first=True)   # x0 @ A_3 (last DMA)
            tail(3)

    nc.compile()
    return nc


def _host_prep(in_proj_base, lora_A, lora_B, A_theta, B_real, B_imag,
               C_real, C_imag, out_proj_w, step_emb):
    W = in_proj_base.astype(np.float64) + 2.0 * (
        lora_B.astype(np.float64) @ lora_A.astype(np.float64))   # [2d, d]
    WinT = np.ascontiguousarray(W.T)                             # [768, 1536]
    WoutT = out_proj_w.astype(np.float64).T                      # [1536, 768]

    th = A_theta.astype(np.float64)
    P = (C_real.astype(np.float64) * B_real.astype(np.float64)
         - C_imag.astype(np.float64) * B_imag.astype(np.float64))
    Q = (C_real.astype(np.float64) * B_imag.astype(np.float64)
         + C_imag.astype(np.float64) * B_real.astype(np.float64))
    g4 = np.stack([
        (P * np.cos(m * th) - Q * np.sin(m * th)).sum(-1).reshape(-1)
        for m in range(NL)
    ])                                                           # [4, 1536]

    A = np.stack([WinT @ (g4[m][:, None] * WoutT) for m in range(NL)])
    s = step_emb.astype(np.float64)
    zb = np.stack([
        sum(s[jj - m] @ A[m] for m in range(jj + 1)) for jj in range(NL)
    ])                                                           # [4, 768]

    # device layout [128, NL, KCH, D]: elem [p][m,k,n] = A[m, 128k+p, n]
    a_dev = np.ascontiguousarray(
        A.reshape(NL, KCH, 128, D).transpose(2, 0, 1, 3)
        .reshape(128, NL * KCH * D)).astype(ml_dtypes.bfloat16)
    zb_dev = zb.reshape(1, NL * D).astype(ml_dtypes.bfloat16)
    s_dev = np.ascontiguousarray(step_emb).astype(np.float32)
    s4T_dev = np.ascontiguousarray(
        s_dev.reshape(NL, KCH, 128).transpose(2, 0, 1)
        .reshape(128, NL * KCH)).astype(ml_dtypes.bfloat16)
    kc_dev = np.array([[
        (D + float((s_dev[jj].astype(np.float64) ** 2).sum())) / D + EPS
        for jj in range(NL)
    ]], dtype=np.float32)
    return a_dev, zb_dev, s_dev, s4T_dev, kc_dev


def kernel(x, in_proj_base, lora_A, lora_B, A_theta, B_real, B_imag,
           C_real, C_imag, out_proj_w, mixer_norm_w, loop_norm_w, step_emb,
           _trace=False):
    x = np.asarray(x, dtype=np.float32)
    a_dev, zb_dev, s_dev, s4T_dev, kc_dev = _host_prep(
        np.asarray(in_proj_base), np.asarray(lora_A), np.asarray(lora_B),
        np.asarray(A_theta), np.asarray(B_real), np.asarray(B_imag),
        np.asarray(C_real), np.asarray(C_imag), np.asarray(out_proj_w),
        np.asarray(step_emb))
    # mixer_norm_w / loop_norm_w are ones per the problem spec; rmsnorm
    # weight multiplies are identity and omitted on device.

    if "nc" not in _CACHE:
        _CACHE["nc"] = build_nc()
    nc = _CACHE["nc"]

    shared = {"a_all": a_dev, "zb4": zb_dev, "s4": s_dev,
              "s4T": s4T_dev, "kc4": kc_dev}
    in_maps = []
    for c in range(NCORES):
        xs = np.ascontiguousarray(x[0, T * c:T * (c + 1), :])    # [128, 768]
        x0T = np.ascontiguousarray(
            xs.T.reshape(KCH, 128, T).transpose(1, 0, 2)
            .reshape(128, KCH * T)).astype(np.float32)
        in_maps.append({**shared, "x0T": x0T})
    res = run_bass_kernel_spmd(nc, in_maps, list(range(NCORES)), trace=_trace)
    out = np.concatenate(
        [np.asarray(res.results[c]["x_out"]) for c in range(NCORES)], axis=0)
    if _trace:
        _CACHE["last_result"] = res
    return out[None, :, :].astype(np.float32)
